# revision 9
# baseline (speedup 1.0000x reference)
"""Batched LoRA Linear on 8 Trainium2 NeuronCores (Bass/Tile).

Computes, for x (32, 512, 4096), adapter_ids (32,), A_all (32, 16, 4096),
B_all (32, 4096, 16), W (4096, 4096), b (4096,):

    out = x @ W.T + b + 2.0 * ((x @ A[aid].T) @ B[aid].T)

Sharding: data-parallel over batch - 4 samples per core; W/b replicated.

Per-core device kernel (bf16 operands, fp32 PSUM accumulation):
  - x is converted to bf16 and kept FULLY resident in SBUF
    ([128, 32 k-tiles, 2048 tokens] = 128 KiB/partition), so W is
    streamed exactly once and there is a single phase (no t-blocks).
  - W-stationary matmuls: for each 128-wide output chunk (oc) and each
    k-tile, ONE weight load feeds 4 matmuls (the 4 x 512-token chunks),
    cutting LDWEIGHTS pressure 4x vs an x-stationary schedule; bf16
    weights additionally get fast-weight-load.
  - PSUM layout is [o_part=128, t=512]; 4 banks accumulate one oc while
    the previous oc's 4 banks drain (bias-add alternating DVE/ACT, then
    one batched DMA per oc).
  - LoRA-1 (inter = A x^T per sample) runs packed in the 4 PE column
    groups (psum partitions 32s..32s+15, one bank per sample), fused
    into oc0's k-loop, so it rides the x-fill phase where the PE has
    idle slots anyway.  inter lands in SBUF exactly when oc0's k-loop
    ends, so every oc - including oc0 - fuses LoRA-2 (one K=16 matmul
    per sample, packed in the 4 PE row groups) into its accumulation
    group before eviction.

Host side only reshapes/transposes/gathers/dtype-converts (no
arithmetic except the exact *2.0 fold into B).
"""

import sys
import types

import numpy as np

# ---------------------------------------------------------------- constants
P = 128
B_SZ = 32            # batch
S = 512              # seq len
D_IN = 4096
D_OUT = 4096
RANK = 16
N_CORES = 8
SPB = B_SZ // N_CORES          # samples per core = 4
T = SPB * S                    # tokens per core = 2048
KT = D_IN // P                 # 32 k-tiles
OC = D_OUT // P                # 32 output chunks of 128
TC = T // S                    # 4 token chunks of 512 (chunk == sample)
SCALING = 2.0

LAST_RESULTS = None            # test harness reads exec_time_ns from here

_COMPILED = {}


def _ensure_axon_hooks_module():
    """If the image's antenv lacks axon_hooks, install a no-op stub so
    run_bass_kernel_spmd(trace=...) degrades gracefully instead of
    raising ImportError."""
    try:
        import antenv.axon_hooks  # noqa: F401
        return
    except ImportError:
        pass
    try:
        import antenv
    except ImportError:
        return
    mod = types.ModuleType("antenv.axon_hooks")
    state = {"hook": None}
    mod.set_axon_ntff_profile_hook = lambda h: state.__setitem__("hook", h)
    mod.get_axon_ntff_profile_hook = lambda: state["hook"]
    sys.modules["antenv.axon_hooks"] = mod
    antenv.axon_hooks = mod


def _build():
    import concourse.bacc as bacc
    import concourse.bass as bass  # noqa: F401
    import concourse.mybir as mybir
    import concourse.tile as tile

    f32 = mybir.dt.float32
    bf16 = mybir.dt.bfloat16
    IDENT = mybir.ActivationFunctionType.Identity

    nc = bacc.Bacc("TRN2", target_bir_lowering=False, debug=False,
                   enable_asserts=False)

    xt_d = nc.dram_tensor("xt", [P, KT, T], bf16, kind="ExternalInput").ap()
    wt_d = nc.dram_tensor("wt", [P, OC, KT, P], bf16, kind="ExternalInput").ap()
    at_d = nc.dram_tensor("at", [P, SPB, KT, RANK], bf16,
                          kind="ExternalInput").ap()
    bt_d = nc.dram_tensor("bt", [P, D_OUT], bf16, kind="ExternalInput").ap()
    bc_d = nc.dram_tensor("bc", [P, OC], f32, kind="ExternalInput").ap()
    out_d = nc.dram_tensor("out", [P, OC, T], f32, kind="ExternalOutput").ap()

    HALF = T // 2                  # 1024 tokens (= 2 token chunks)
    EARLY = 3                      # ocs 0..2 run phase-split over halves
    JOIN = {0: 0, 1: 6, 2: 10}     # kt at which each early oc joins

    with tile.TileContext(nc) as tc:
        with (
            tc.tile_pool(name="xt", bufs=KT) as xt_pool,
            tc.tile_pool(name="wt", bufs=4) as wt_pool,
            tc.tile_pool(name="misc", bufs=1) as misc_pool,
            tc.tile_pool(name="ob", bufs=2) as out_pool,
            tc.tile_pool(name="oh", bufs=3) as oh_pool,
            tc.tile_pool(name="ps", bufs=8, space="PSUM") as ps_pool,
        ):
            # ---- prologue DMAs ----
            # sync queue order: wt0 first half (gates the very first
            # matmul, ~1.5us), at (gates lora1), wt0 second half,
            # wt1/wt2 (early ocs), then bt/bc (needed only at fill end).
            wts = [wt_pool.tile([P, KT, P], bf16, name=f"wt_{oc}", tag="wt")
                   for oc in range(EARLY)]
            nc.sync.dma_start(wts[0][:, 0:KT // 2, :],
                              wt_d[:, 0, 0:KT // 2, :])
            at_sb = misc_pool.tile([P, SPB, KT, RANK], bf16,
                                   name="at_sb", tag="at")
            nc.sync.dma_start(at_sb[:], at_d[:])
            nc.sync.dma_start(wts[0][:, KT // 2:, :], wt_d[:, 0, KT // 2:, :])
            nc.sync.dma_start(wts[1][:], wt_d[:, 1])
            nc.sync.dma_start(wts[2][:], wt_d[:, 2])
            bt_sb = misc_pool.tile([P, D_OUT], bf16, name="bt_sb", tag="bt")
            nc.sync.dma_start(bt_sb[:], bt_d[:])
            bc_sb = misc_pool.tile([P, OC], f32, name="bc_sb", tag="bc")
            nc.sync.dma_start(bc_sb[:], bc_d[:])
            inter_sb = misc_pool.tile([P, S], bf16, name="inter_sb",
                                      tag="inter")

            # x: loaded in token halves (all kts' first halves, then all
            # second halves) so the first half-fill already exposes a
            # full accumulation problem; split across gpsimd/scalar
            # queues (the fill is chip-HBM-bound).
            xqueues = [nc.gpsimd, nc.scalar]
            xts = [xt_pool.tile([P, T], bf16, name=f"xt_{kt}", tag="xt")
                   for kt in range(KT)]
            for h in range(2):
                for kt in range(KT):
                    xqueues[kt % 2].dma_start(
                        xts[kt][:, h * HALF:(h + 1) * HALF],
                        xt_d[:, kt, h * HALF:(h + 1) * HALF])

            def evict_chunk(o_t, off, psum, oc, use_act):
                if use_act:
                    nc.scalar.activation(o_t[:, off:off + S], psum[:],
                                         IDENT, bias=bc_sb[:, oc:oc + 1])
                else:
                    nc.vector.tensor_scalar_add(o_t[:, off:off + S],
                                                psum[:], bc_sb[:, oc:oc + 1])

            def emit_lora2(psum, oc, s):
                nc.tensor.matmul(
                    psum[:, :],
                    bt_sb[32 * s:32 * s + RANK, oc * P:(oc + 1) * P],
                    inter_sb[32 * s:32 * s + RANK, :],
                    start=False, stop=True, tile_position=(32 * s, 0))

            # ---- phases over token halves for ocs 0..2 + lora1 ----
            for h in range(2):
                tcs = (0, 1) if h == 0 else (2, 3)
                ps_e = [[ps_pool.tile([P, S], f32, name=f"pse{h}_{oc}_{t}",
                                      tag="ps") for t in range(2)]
                        for oc in range(EARLY)]
                ps_l = [ps_pool.tile([P, S], f32, name=f"psl_{s}", tag="ps")
                        for s in tcs]
                for kt in range(KT):
                    for oc in range(EARLY):
                        if h == 0 and kt < JOIN[oc]:
                            continue
                        st = JOIN[oc] if h == 0 else 0
                        for i, t in enumerate(tcs):
                            nc.tensor.matmul(
                                ps_e[oc][i][:, :],
                                wts[oc][:, kt, :],
                                xts[kt][:, t * S:(t + 1) * S],
                                start=(kt == st), stop=False)
                    for i, s in enumerate(tcs):
                        nc.tensor.matmul(
                            ps_l[i][32 * s:32 * s + RANK, :],
                            at_sb[:, s, kt, :],
                            xts[kt][:, s * S:(s + 1) * S],
                            start=(kt == 0), stop=(kt == KT - 1),
                            tile_position=(0, 32 * s))
                if h == 0:
                    # backfill the join-delayed kts
                    for oc in range(1, EARLY):
                        for kt in range(JOIN[oc]):
                            for i, t in enumerate(tcs):
                                nc.tensor.matmul(
                                    ps_e[oc][i][:, :],
                                    wts[oc][:, kt, :],
                                    xts[kt][:, t * S:(t + 1) * S],
                                    start=False, stop=False)
                # inter (bf16) at partitions 32s..32s+15; frees lora banks
                for i, s in enumerate(tcs):
                    nc.vector.tensor_copy(inter_sb[32 * s:32 * s + RANK, :],
                                          ps_l[i][32 * s:32 * s + RANK, :])
                # fused lora2 + eviction + store per early oc
                for oc in range(EARLY):
                    oh_t = oh_pool.tile([P, HALF], f32, name=f"oh{h}_{oc}",
                                        tag="oh")
                    for i, t in enumerate(tcs):
                        emit_lora2(ps_e[oc][i], oc, t)
                    for i, t in enumerate(tcs):
                        evict_chunk(oh_t, i * S, ps_e[oc][i], oc, i % 2 == 1)
                    nc.scalar.dma_start(
                        out_d[:, oc, h * HALF:(h + 1) * HALF], oh_t[:])

            # ---- ocs 3..31: standard fused loop ----
            for oc in range(EARLY, OC):
                wt_t = wt_pool.tile([P, KT, P], bf16, name=f"wt_{oc}",
                                    tag="wt")
                nc.sync.dma_start(wt_t[:], wt_d[:, oc])
                psums = [ps_pool.tile([P, S], f32, name=f"ps_{oc}_{t}",
                                      tag="ps")
                         for t in range(TC)]
                for kt in range(KT):
                    for t in range(TC):
                        nc.tensor.matmul(
                            psums[t][:, :],
                            wt_t[:, kt, :],
                            xts[kt][:, t * S:(t + 1) * S],
                            start=(kt == 0), stop=False)
                for t in range(TC):
                    emit_lora2(psums[t], oc, t)
                o_t = out_pool.tile([P, T], f32, name=f"o_{oc}", tag="o")
                last = (oc == OC - 1)
                for t in range(TC):
                    evict_chunk(o_t, t * S, psums[t], oc, t % 2 == 1)
                    if last:
                        # per-chunk DMAs so the final chunk's store does
                        # not wait for all four evictions
                        nc.scalar.dma_start(
                            out_d[:, oc, t * S:(t + 1) * S],
                            o_t[:, t * S:(t + 1) * S])
                if not last:
                    nc.scalar.dma_start(out_d[:, oc], o_t[:])

    nc.compile()
    return nc


def _get_compiled():
    if "nc" not in _COMPILED:
        _COMPILED["nc"] = _build()
    return _COMPILED["nc"]


def kernel(x, adapter_ids, A_all, B_all, W, b):
    global LAST_RESULTS
    _ensure_axon_hooks_module()
    import ml_dtypes
    from concourse.bass_utils import run_bass_kernel_spmd

    bf16 = ml_dtypes.bfloat16

    x = np.asarray(x, dtype=np.float32)
    adapter_ids = np.asarray(adapter_ids)
    A_all = np.asarray(A_all, dtype=np.float32)
    B_all = np.asarray(B_all, dtype=np.float32)
    W = np.asarray(W, dtype=np.float32)
    b = np.asarray(b, dtype=np.float32)

    nc = _get_compiled()

    # ---- host-side layout prep (reshape/transpose/gather/dtype only) ----
    # wt[p, oc, kt, o'] = W[oc*128+o', kt*128+p]
    wt_np = np.ascontiguousarray(
        W.astype(bf16).reshape(OC, P, KT, P).transpose(3, 0, 2, 1))
    # bc[p, oc] = b[oc*128+p]
    bc_np = np.ascontiguousarray(b.reshape(OC, P).T)

    A_batch = A_all[adapter_ids]              # (B, R, D_IN)
    B_batch = B_all[adapter_ids] * SCALING    # (B, D_OUT, R) - exact *2 fold

    in_maps = []
    for c in range(N_CORES):
        xs = x[c * SPB:(c + 1) * SPB].reshape(T, D_IN).astype(bf16)
        # xt[p, kt, t] = x_core[t, kt*128+p]
        xt_np = np.ascontiguousarray(
            xs.reshape(T, KT, P).transpose(2, 1, 0))
        A_c = A_batch[c * SPB:(c + 1) * SPB].astype(bf16)   # (SPB, R, D_IN)
        # at[p, s, kt, r] = A_c[s, r, kt*128+p]
        at_np = np.ascontiguousarray(
            A_c.reshape(SPB, RANK, KT, P).transpose(3, 0, 2, 1))
        B_c = B_batch[c * SPB:(c + 1) * SPB].astype(bf16)   # (SPB, D_OUT, R)
        # bt[32s+r, o] = 2*B_c[s][o, r]
        bt_np = np.zeros((P, D_OUT), dtype=bf16)
        for s in range(SPB):
            bt_np[32 * s:32 * s + RANK, :] = B_c[s].T
        in_maps.append({
            "xt": xt_np, "wt": wt_np, "at": at_np, "bt": bt_np,
            "bc": bc_np,
        })

    res = run_bass_kernel_spmd(nc, in_maps, core_ids=list(range(N_CORES)))
    LAST_RESULTS = res

    out = np.empty((B_SZ, S, D_OUT), dtype=np.float32)
    for c in range(N_CORES):
        oc_np = res.results[c]["out"]              # [p, oc, t]
        out[c * SPB:(c + 1) * SPB] = (
            oc_np.transpose(2, 1, 0).reshape(T, D_OUT)
            .reshape(SPB, S, D_OUT))
    return out


# revision 10
# speedup vs baseline: 1.0142x; 1.0142x over previous
"""Batched LoRA Linear on 8 Trainium2 NeuronCores (Bass/Tile).

Computes, for x (32, 512, 4096), adapter_ids (32,), A_all (32, 16, 4096),
B_all (32, 4096, 16), W (4096, 4096), b (4096,):

    out = x @ W.T + b + 2.0 * ((x @ A[aid].T) @ B[aid].T)

Sharding: data-parallel over batch - 4 samples per core; W/b replicated.

Per-core device kernel (bf16 operands, fp32 PSUM accumulation):
  - x is converted to bf16 and kept FULLY resident in SBUF
    ([128, 32 k-tiles, 2048 tokens] = 128 KiB/partition), so W is
    streamed exactly once and there is a single phase (no t-blocks).
  - W-stationary matmuls: for each 128-wide output chunk (oc) and each
    k-tile, ONE weight load feeds 4 matmuls (the 4 x 512-token chunks),
    cutting LDWEIGHTS pressure 4x vs an x-stationary schedule; bf16
    weights additionally get fast-weight-load.
  - PSUM layout is [o_part=128, t=512]; 4 banks accumulate one oc while
    the previous oc's 4 banks drain (bias-add alternating DVE/ACT, then
    one batched DMA per oc).
  - LoRA-1 (inter = A x^T per sample) runs packed in the 4 PE column
    groups (psum partitions 32s..32s+15, one bank per sample), fused
    into oc0's k-loop, so it rides the x-fill phase where the PE has
    idle slots anyway.  inter lands in SBUF exactly when oc0's k-loop
    ends, so every oc - including oc0 - fuses LoRA-2 (one K=16 matmul
    per sample, packed in the 4 PE row groups) into its accumulation
    group before eviction.

Host side only reshapes/transposes/gathers/dtype-converts (no
arithmetic except the exact *2.0 fold into B).
"""

import sys
import types

import numpy as np

# ---------------------------------------------------------------- constants
P = 128
B_SZ = 32            # batch
S = 512              # seq len
D_IN = 4096
D_OUT = 4096
RANK = 16
N_CORES = 8
SPB = B_SZ // N_CORES          # samples per core = 4
T = SPB * S                    # tokens per core = 2048
KT = D_IN // P                 # 32 k-tiles
OC = D_OUT // P                # 32 output chunks of 128
TC = T // S                    # 4 token chunks of 512 (chunk == sample)
SCALING = 2.0

LAST_RESULTS = None            # test harness reads exec_time_ns from here

_COMPILED = {}


def _ensure_axon_hooks_module():
    """If the image's antenv lacks axon_hooks, install a no-op stub so
    run_bass_kernel_spmd(trace=...) degrades gracefully instead of
    raising ImportError."""
    try:
        import antenv.axon_hooks  # noqa: F401
        return
    except ImportError:
        pass
    try:
        import antenv
    except ImportError:
        return
    mod = types.ModuleType("antenv.axon_hooks")
    state = {"hook": None}
    mod.set_axon_ntff_profile_hook = lambda h: state.__setitem__("hook", h)
    mod.get_axon_ntff_profile_hook = lambda: state["hook"]
    sys.modules["antenv.axon_hooks"] = mod
    antenv.axon_hooks = mod


def _build():
    import concourse.bacc as bacc
    import concourse.bass as bass  # noqa: F401
    import concourse.mybir as mybir
    import concourse.tile as tile

    f32 = mybir.dt.float32
    bf16 = mybir.dt.bfloat16
    IDENT = mybir.ActivationFunctionType.Identity

    nc = bacc.Bacc("TRN2", target_bir_lowering=False, debug=False,
                   enable_asserts=False)

    xt_d = nc.dram_tensor("xt", [P, KT, T], bf16, kind="ExternalInput").ap()
    wt_d = nc.dram_tensor("wt", [P, OC, KT, P], bf16, kind="ExternalInput").ap()
    at_d = nc.dram_tensor("at", [P, SPB, KT, RANK], bf16,
                          kind="ExternalInput").ap()
    bt_d = nc.dram_tensor("bt", [P, D_OUT], bf16, kind="ExternalInput").ap()
    bc_d = nc.dram_tensor("bc", [P, OC], f32, kind="ExternalInput").ap()
    out_d = nc.dram_tensor("out", [P, OC, T], f32, kind="ExternalOutput").ap()

    with tile.TileContext(nc) as tc:
        with (
            tc.tile_pool(name="xt", bufs=KT) as xt_pool,
            tc.tile_pool(name="wt", bufs=3) as wt_pool,
            tc.tile_pool(name="misc", bufs=1) as misc_pool,
            tc.tile_pool(name="ob", bufs=2) as out_pool,
            tc.tile_pool(name="ps", bufs=8, space="PSUM") as ps_pool,
        ):
            # ---- prologue DMAs ----
            # sync queue order: wt0 first half (gates the very first
            # matmul, ~1.5us), at (gates lora1), wt0 second half, wt1,
            # then bt/bc (needed only at fill end).
            wt0 = wt_pool.tile([P, KT, P], bf16, name="wt_0", tag="wt")
            nc.sync.dma_start(wt0[:, 0:KT // 2, :], wt_d[:, 0, 0:KT // 2, :])
            at_sb = misc_pool.tile([P, SPB, KT, RANK], bf16,
                                   name="at_sb", tag="at")
            nc.sync.dma_start(at_sb[:], at_d[:])
            nc.sync.dma_start(wt0[:, KT // 2:, :], wt_d[:, 0, KT // 2:, :])
            wt1 = wt_pool.tile([P, KT, P], bf16, name="wt_1", tag="wt")
            nc.sync.dma_start(wt1[:], wt_d[:, 1])
            bt_sb = misc_pool.tile([P, D_OUT], bf16, name="bt_sb", tag="bt")
            nc.sync.dma_start(bt_sb[:], bt_d[:])
            bc_sb = misc_pool.tile([P, OC], f32, name="bc_sb", tag="bc")
            nc.sync.dma_start(bc_sb[:], bc_d[:])
            inter_sb = misc_pool.tile([P, S], bf16, name="inter_sb",
                                      tag="inter")

            # x: full-residency load, split across the gpsimd and scalar
            # DMA queues (the fill is chip-HBM-bound; 2 queues suffice).
            xqueues = [nc.gpsimd, nc.scalar]
            xts = []
            for kt in range(KT):
                xt_t = xt_pool.tile([P, T], bf16, name=f"xt_{kt}", tag="xt")
                xqueues[kt % 2].dma_start(xt_t[:], xt_d[:, kt])
                xts.append(xt_t)

            # lora1 psums: one bank per sample, output in PE col group s
            ps_l = [ps_pool.tile([P, S], f32, name=f"psl_{s}", tag="ps")
                    for s in range(SPB)]

            def emit_oc(oc, wt_t, fuse_lora1):
                psums = [ps_pool.tile([P, S], f32, name=f"ps_{oc}_{t}",
                                      tag="ps")
                         for t in range(TC)]
                for kt in range(KT):
                    for t in range(TC):
                        nc.tensor.matmul(
                            psums[t][:, :],
                            wt_t[:, kt, :],
                            xts[kt][:, t * S:(t + 1) * S],
                            start=(kt == 0), stop=False)
                    if fuse_lora1:
                        for s in range(SPB):
                            nc.tensor.matmul(
                                ps_l[s][32 * s:32 * s + RANK, :],
                                at_sb[:, s, kt, :],
                                xts[kt][:, s * S:(s + 1) * S],
                                start=(kt == 0), stop=(kt == KT - 1),
                                tile_position=(0, 32 * s))
                if fuse_lora1:
                    # inter (bf16) at partitions 32s..32s+15; frees banks
                    for s in range(SPB):
                        nc.vector.tensor_copy(
                            inter_sb[32 * s:32 * s + RANK, :],
                            ps_l[s][32 * s:32 * s + RANK, :])
                # fused lora2: 4 K=16 matmuls in the 4 PE row groups
                for t in range(TC):
                    s = t
                    nc.tensor.matmul(
                        psums[t][:, :],
                        bt_sb[32 * s:32 * s + RANK, oc * P:(oc + 1) * P],
                        inter_sb[32 * s:32 * s + RANK, :],
                        start=False, stop=True, tile_position=(32 * s, 0))
                # eviction: bias-add split across DVE/ACT, batched DMA
                o_t = out_pool.tile([P, T], f32, name=f"o_{oc}", tag="o")
                last = (oc == OC - 1)
                for t in range(TC):
                    if t % 2 == 0:
                        nc.vector.tensor_scalar_add(
                            o_t[:, t * S:(t + 1) * S], psums[t][:],
                            bc_sb[:, oc:oc + 1])
                    else:
                        nc.scalar.activation(o_t[:, t * S:(t + 1) * S],
                                             psums[t][:], IDENT,
                                             bias=bc_sb[:, oc:oc + 1])
                    if last:
                        # per-chunk DMAs so the final chunk's store does
                        # not wait for all four evictions
                        nc.scalar.dma_start(
                            out_d[:, oc, t * S:(t + 1) * S],
                            o_t[:, t * S:(t + 1) * S])
                if not last:
                    nc.scalar.dma_start(out_d[:, oc], o_t[:])

            emit_oc(0, wt0, fuse_lora1=True)
            for oc in range(1, OC):
                if oc >= 2:
                    wt_t = wt_pool.tile([P, KT, P], bf16, name=f"wt_{oc}",
                                        tag="wt")
                    nc.sync.dma_start(wt_t[:], wt_d[:, oc])
                else:
                    wt_t = wt1
                emit_oc(oc, wt_t, fuse_lora1=False)

    nc.compile()
    return nc


def _get_compiled():
    if "nc" not in _COMPILED:
        _COMPILED["nc"] = _build()
    return _COMPILED["nc"]


def kernel(x, adapter_ids, A_all, B_all, W, b):
    global LAST_RESULTS
    _ensure_axon_hooks_module()
    import ml_dtypes
    from concourse.bass_utils import run_bass_kernel_spmd

    bf16 = ml_dtypes.bfloat16

    x = np.asarray(x, dtype=np.float32)
    adapter_ids = np.asarray(adapter_ids)
    A_all = np.asarray(A_all, dtype=np.float32)
    B_all = np.asarray(B_all, dtype=np.float32)
    W = np.asarray(W, dtype=np.float32)
    b = np.asarray(b, dtype=np.float32)

    nc = _get_compiled()

    # ---- host-side layout prep (reshape/transpose/gather/dtype only) ----
    # wt[p, oc, kt, o'] = W[oc*128+o', kt*128+p]
    wt_np = np.ascontiguousarray(
        W.astype(bf16).reshape(OC, P, KT, P).transpose(3, 0, 2, 1))
    # bc[p, oc] = b[oc*128+p]
    bc_np = np.ascontiguousarray(b.reshape(OC, P).T)

    A_batch = A_all[adapter_ids]              # (B, R, D_IN)
    B_batch = B_all[adapter_ids] * SCALING    # (B, D_OUT, R) - exact *2 fold

    in_maps = []
    for c in range(N_CORES):
        xs = x[c * SPB:(c + 1) * SPB].reshape(T, D_IN).astype(bf16)
        # xt[p, kt, t] = x_core[t, kt*128+p]
        xt_np = np.ascontiguousarray(
            xs.reshape(T, KT, P).transpose(2, 1, 0))
        A_c = A_batch[c * SPB:(c + 1) * SPB].astype(bf16)   # (SPB, R, D_IN)
        # at[p, s, kt, r] = A_c[s, r, kt*128+p]
        at_np = np.ascontiguousarray(
            A_c.reshape(SPB, RANK, KT, P).transpose(3, 0, 2, 1))
        B_c = B_batch[c * SPB:(c + 1) * SPB].astype(bf16)   # (SPB, D_OUT, R)
        # bt[32s+r, o] = 2*B_c[s][o, r]
        bt_np = np.zeros((P, D_OUT), dtype=bf16)
        for s in range(SPB):
            bt_np[32 * s:32 * s + RANK, :] = B_c[s].T
        in_maps.append({
            "xt": xt_np, "wt": wt_np, "at": at_np, "bt": bt_np,
            "bc": bc_np,
        })

    res = run_bass_kernel_spmd(nc, in_maps, core_ids=list(range(N_CORES)))
    LAST_RESULTS = res

    out = np.empty((B_SZ, S, D_OUT), dtype=np.float32)
    for c in range(N_CORES):
        oc_np = res.results[c]["out"]              # [p, oc, t]
        out[c * SPB:(c + 1) * SPB] = (
            oc_np.transpose(2, 1, 0).reshape(T, D_OUT)
            .reshape(SPB, S, D_OUT))
    return out


# revision 11
# speedup vs baseline: 1.0289x; 1.0145x over previous
"""Batched LoRA Linear on 8 Trainium2 NeuronCores (Bass/Tile).

Computes, for x (32, 512, 4096), adapter_ids (32,), A_all (32, 16, 4096),
B_all (32, 4096, 16), W (4096, 4096), b (4096,):

    out = x @ W.T + b + 2.0 * ((x @ A[aid].T) @ B[aid].T)

Sharding: data-parallel over batch - 4 samples per core; W/b replicated.

Per-core device kernel (bf16 operands, fp32 PSUM accumulation):
  - x is converted to bf16 and kept FULLY resident in SBUF
    ([128, 32 k-tiles, 2048 tokens] = 128 KiB/partition), so W is
    streamed exactly once and there is a single phase (no t-blocks).
  - W-stationary matmuls: for each 128-wide output chunk (oc) and each
    k-tile, ONE weight load feeds 4 matmuls (the 4 x 512-token chunks),
    cutting LDWEIGHTS pressure 4x vs an x-stationary schedule; bf16
    weights additionally get fast-weight-load.
  - PSUM layout is [o_part=128, t=512]; 4 banks accumulate one oc while
    the previous oc's 4 banks drain (bias-add alternating DVE/ACT, then
    one batched DMA per oc).
  - LoRA-1 (inter = A x^T per sample) runs packed in the 4 PE column
    groups (psum partitions 32s..32s+15, one bank per sample), fused
    into oc0's k-loop, so it rides the x-fill phase where the PE has
    idle slots anyway.  inter lands in SBUF exactly when oc0's k-loop
    ends, so every oc - including oc0 - fuses LoRA-2 (one K=16 matmul
    per sample, packed in the 4 PE row groups) into its accumulation
    group before eviction.

Host side only reshapes/transposes/gathers/dtype-converts (no
arithmetic except the exact *2.0 fold into B).
"""

import sys
import types

import numpy as np

# ---------------------------------------------------------------- constants
P = 128
B_SZ = 32            # batch
S = 512              # seq len
D_IN = 4096
D_OUT = 4096
RANK = 16
N_CORES = 8
SPB = B_SZ // N_CORES          # samples per core = 4
T = SPB * S                    # tokens per core = 2048
KT = D_IN // P                 # 32 k-tiles
OC = D_OUT // P                # 32 output chunks of 128
TC = T // S                    # 4 token chunks of 512 (chunk == sample)
SCALING = 2.0

LAST_RESULTS = None            # test harness reads exec_time_ns from here

_COMPILED = {}


def _ensure_axon_hooks_module():
    """If the image's antenv lacks axon_hooks, install a no-op stub so
    run_bass_kernel_spmd(trace=...) degrades gracefully instead of
    raising ImportError."""
    try:
        import antenv.axon_hooks  # noqa: F401
        return
    except ImportError:
        pass
    try:
        import antenv
    except ImportError:
        return
    mod = types.ModuleType("antenv.axon_hooks")
    state = {"hook": None}
    mod.set_axon_ntff_profile_hook = lambda h: state.__setitem__("hook", h)
    mod.get_axon_ntff_profile_hook = lambda: state["hook"]
    sys.modules["antenv.axon_hooks"] = mod
    antenv.axon_hooks = mod


def _build():
    import concourse.bacc as bacc
    import concourse.bass as bass  # noqa: F401
    import concourse.mybir as mybir
    import concourse.tile as tile

    f32 = mybir.dt.float32
    bf16 = mybir.dt.bfloat16
    IDENT = mybir.ActivationFunctionType.Identity

    nc = bacc.Bacc("TRN2", target_bir_lowering=False, debug=False,
                   enable_asserts=False)

    xt_d = nc.dram_tensor("xt", [P, KT, T], bf16, kind="ExternalInput").ap()
    wt_d = nc.dram_tensor("wt", [P, OC, KT, P], bf16, kind="ExternalInput").ap()
    at_d = nc.dram_tensor("at", [P, SPB, KT, RANK], bf16,
                          kind="ExternalInput").ap()
    bt_d = nc.dram_tensor("bt", [P, D_OUT], bf16, kind="ExternalInput").ap()
    bc_d = nc.dram_tensor("bc", [P, OC], f32, kind="ExternalInput").ap()
    out_d = nc.dram_tensor("out", [P, OC, T], f32, kind="ExternalOutput").ap()

    with tile.TileContext(nc) as tc:
        with (
            tc.tile_pool(name="xt", bufs=KT) as xt_pool,
            tc.tile_pool(name="wt", bufs=3) as wt_pool,
            tc.tile_pool(name="misc", bufs=1) as misc_pool,
            tc.tile_pool(name="ob", bufs=2) as out_pool,
            tc.tile_pool(name="ps", bufs=8, space="PSUM") as ps_pool,
        ):
            # ---- prologue DMAs ----
            # sync queue order: wt0 first half (gates the very first
            # matmul, ~1.5us), at (gates lora1), wt0 second half, wt1,
            # then bt/bc (needed only at fill end).
            wt0 = wt_pool.tile([P, KT, P], bf16, name="wt_0", tag="wt")
            nc.sync.dma_start(wt0[:, 0:KT // 2, :], wt_d[:, 0, 0:KT // 2, :])
            at_sb = misc_pool.tile([P, SPB, KT, RANK], bf16,
                                   name="at_sb", tag="at")
            nc.sync.dma_start(at_sb[:], at_d[:])
            nc.sync.dma_start(wt0[:, KT // 2:, :], wt_d[:, 0, KT // 2:, :])
            wt1 = wt_pool.tile([P, KT, P], bf16, name="wt_1", tag="wt")
            nc.sync.dma_start(wt1[:], wt_d[:, 1])
            bt_sb = misc_pool.tile([P, D_OUT], bf16, name="bt_sb", tag="bt")
            nc.sync.dma_start(bt_sb[:], bt_d[:])
            bc_sb = misc_pool.tile([P, OC], f32, name="bc_sb", tag="bc")
            nc.sync.dma_start(bc_sb[:], bc_d[:])
            inter_sb = misc_pool.tile([P, S], bf16, name="inter_sb",
                                      tag="inter")

            # x: full-residency load, split across the gpsimd and scalar
            # DMA queues (the fill is chip-HBM-bound; 2 queues suffice).
            xqueues = [nc.gpsimd, nc.scalar]
            xts = []
            for kt in range(KT):
                xt_t = xt_pool.tile([P, T], bf16, name=f"xt_{kt}", tag="xt")
                xqueues[kt % 2].dma_start(xt_t[:], xt_d[:, kt])
                xts.append(xt_t)

            JOIN = 6   # kt at which oc1's first half joins the fill loop

            def emit_lora2(psum, oc, s):
                nc.tensor.matmul(
                    psum[:, :],
                    bt_sb[32 * s:32 * s + RANK, oc * P:(oc + 1) * P],
                    inter_sb[32 * s:32 * s + RANK, :],
                    start=False, stop=True, tile_position=(32 * s, 0))

            def emit_evict(psums, oc, last=False):
                o_t = out_pool.tile([P, T], f32, name=f"o_{oc}", tag="o")
                for t in range(TC):
                    if t % 2 == 0:
                        nc.vector.tensor_scalar_add(
                            o_t[:, t * S:(t + 1) * S], psums[t][:],
                            bc_sb[:, oc:oc + 1])
                    else:
                        nc.scalar.activation(o_t[:, t * S:(t + 1) * S],
                                             psums[t][:], IDENT,
                                             bias=bc_sb[:, oc:oc + 1])
                    if last:
                        # per-chunk DMAs so the final chunk's store does
                        # not wait for all four evictions
                        nc.scalar.dma_start(
                            out_d[:, oc, t * S:(t + 1) * S],
                            o_t[:, t * S:(t + 1) * S])
                if not last:
                    nc.scalar.dma_start(out_d[:, oc], o_t[:])

            # lora1: 4 samples packed in the 4 PE col groups, TWO banks
            # (s0/s2 share bank A at partitions 0-15/64-79, s1/s3 bank B
            # at 32-47/96-111).  Each group opens with its own start=True,
            # which clears only its own region's has_written bits.
            psA = ps_pool.tile([P, S], f32, name="psA", tag="ps")
            psB = ps_pool.tile([P, S], f32, name="psB", tag="ps")
            lbank = [psA, psB, psA, psB]
            ps0 = [ps_pool.tile([P, S], f32, name=f"ps_0_{t}", tag="ps")
                   for t in range(TC)]
            ps1 = [ps_pool.tile([P, S], f32, name=f"ps_1_{t}", tag="ps")
                   for t in range(2)]

            # ---- fill loop: oc0 + lora1 (all 4 samples) + oc1 tc0/1 ----
            for kt in range(KT):
                for t in range(TC):
                    nc.tensor.matmul(
                        ps0[t][:, :],
                        wt0[:, kt, :],
                        xts[kt][:, t * S:(t + 1) * S],
                        start=(kt == 0), stop=False)
                for s in range(SPB):
                    nc.tensor.matmul(
                        lbank[s][32 * s:32 * s + RANK, :],
                        at_sb[:, s, kt, :],
                        xts[kt][:, s * S:(s + 1) * S],
                        start=(kt == 0), stop=(kt == KT - 1),
                        tile_position=(0, 32 * s), skip_group_check=True)
                if kt >= JOIN:
                    for t in range(2):
                        nc.tensor.matmul(
                            ps1[t][:, :],
                            wt1[:, kt, :],
                            xts[kt][:, t * S:(t + 1) * S],
                            start=(kt == JOIN), stop=False)

            # inter (bf16) at partitions 32s..32s+15; frees banks A/B
            for s in range(SPB):
                nc.vector.tensor_copy(inter_sb[32 * s:32 * s + RANK, :],
                                      lbank[s][32 * s:32 * s + RANK, :])

            # oc0: fused lora2 + eviction + store
            for t in range(TC):
                emit_lora2(ps0[t], 0, t)
            emit_evict(ps0, 0)

            # oc1: tc2/3 full k-loops, backfill tc0/1 kts 0..5, lora2,
            # eviction
            ps1 += [ps_pool.tile([P, S], f32, name=f"ps_1_{t}", tag="ps")
                    for t in range(2, TC)]
            for kt in range(KT):
                for t in range(2, TC):
                    nc.tensor.matmul(
                        ps1[t][:, :],
                        wt1[:, kt, :],
                        xts[kt][:, t * S:(t + 1) * S],
                        start=(kt == 0), stop=False)
            for kt in range(JOIN):
                for t in range(2):
                    nc.tensor.matmul(
                        ps1[t][:, :],
                        wt1[:, kt, :],
                        xts[kt][:, t * S:(t + 1) * S],
                        start=False, stop=False)
            for t in range(TC):
                emit_lora2(ps1[t], 1, t)
            emit_evict(ps1, 1)

            # ---- oc 2..31: standard fused loop ----
            for oc in range(2, OC):
                wt_t = wt_pool.tile([P, KT, P], bf16, name=f"wt_{oc}",
                                    tag="wt")
                nc.sync.dma_start(wt_t[:], wt_d[:, oc])
                psums = [ps_pool.tile([P, S], f32, name=f"ps_{oc}_{t}",
                                      tag="ps")
                         for t in range(TC)]
                for kt in range(KT):
                    for t in range(TC):
                        nc.tensor.matmul(
                            psums[t][:, :],
                            wt_t[:, kt, :],
                            xts[kt][:, t * S:(t + 1) * S],
                            start=(kt == 0), stop=False)
                for t in range(TC):
                    emit_lora2(psums[t], oc, t)
                emit_evict(psums, oc, last=(oc == OC - 1))

    nc.compile()
    return nc


def _get_compiled():
    if "nc" not in _COMPILED:
        _COMPILED["nc"] = _build()
    return _COMPILED["nc"]


def kernel(x, adapter_ids, A_all, B_all, W, b):
    global LAST_RESULTS
    _ensure_axon_hooks_module()
    import ml_dtypes
    from concourse.bass_utils import run_bass_kernel_spmd

    bf16 = ml_dtypes.bfloat16

    x = np.asarray(x, dtype=np.float32)
    adapter_ids = np.asarray(adapter_ids)
    A_all = np.asarray(A_all, dtype=np.float32)
    B_all = np.asarray(B_all, dtype=np.float32)
    W = np.asarray(W, dtype=np.float32)
    b = np.asarray(b, dtype=np.float32)

    nc = _get_compiled()

    # ---- host-side layout prep (reshape/transpose/gather/dtype only) ----
    # wt[p, oc, kt, o'] = W[oc*128+o', kt*128+p]
    wt_np = np.ascontiguousarray(
        W.astype(bf16).reshape(OC, P, KT, P).transpose(3, 0, 2, 1))
    # bc[p, oc] = b[oc*128+p]
    bc_np = np.ascontiguousarray(b.reshape(OC, P).T)

    A_batch = A_all[adapter_ids]              # (B, R, D_IN)
    B_batch = B_all[adapter_ids] * SCALING    # (B, D_OUT, R) - exact *2 fold

    in_maps = []
    for c in range(N_CORES):
        xs = x[c * SPB:(c + 1) * SPB].reshape(T, D_IN).astype(bf16)
        # xt[p, kt, t] = x_core[t, kt*128+p]
        xt_np = np.ascontiguousarray(
            xs.reshape(T, KT, P).transpose(2, 1, 0))
        A_c = A_batch[c * SPB:(c + 1) * SPB].astype(bf16)   # (SPB, R, D_IN)
        # at[p, s, kt, r] = A_c[s, r, kt*128+p]
        at_np = np.ascontiguousarray(
            A_c.reshape(SPB, RANK, KT, P).transpose(3, 0, 2, 1))
        B_c = B_batch[c * SPB:(c + 1) * SPB].astype(bf16)   # (SPB, D_OUT, R)
        # bt[32s+r, o] = 2*B_c[s][o, r]
        bt_np = np.zeros((P, D_OUT), dtype=bf16)
        for s in range(SPB):
            bt_np[32 * s:32 * s + RANK, :] = B_c[s].T
        in_maps.append({
            "xt": xt_np, "wt": wt_np, "at": at_np, "bt": bt_np,
            "bc": bc_np,
        })

    res = run_bass_kernel_spmd(nc, in_maps, core_ids=list(range(N_CORES)))
    LAST_RESULTS = res

    out = np.empty((B_SZ, S, D_OUT), dtype=np.float32)
    for c in range(N_CORES):
        oc_np = res.results[c]["out"]              # [p, oc, t]
        out[c * SPB:(c + 1) * SPB] = (
            oc_np.transpose(2, 1, 0).reshape(T, D_OUT)
            .reshape(SPB, S, D_OUT))
    return out
